# revision 1
# baseline (speedup 1.0000x reference)
"""Trainium2 Bass kernel for nn_DglAggregator (GNN message passing).

Strategy (8 NeuronCores, SPMD, one uniform program, per-core data):
- Targets are partitioned across cores balanced by stage-1 edge count; each
  core owns its targets' items and ALL stage-1 edges pointing at those items,
  so no cross-core communication is needed.
- Stage 1 (item->item segment softmax + weighted sum): items laid out in
  contiguous "islot" order; windows = consecutive islot ranges (<=128 islots,
  <=1024 edges); 8 windows per batch (8192 edge ranks). Per batch the edge
  source rows come from ONE bf16 dma_gather ([edge, d] rank order) out of a
  per-batch deduplicated region (<=8192 rows, int16-addressable) of a
  host-relaid table; the transposed layout [d, edge] needed by the score
  matmul is produced on-chip (PE transposes through PSUM + Act/DVE copies).
  Scores: S[e,s] = Xs_e . (h_v[dst_s] * pi) on TensorE against per-window
  dst-slot columns (no per-edge dst gather; pi is folded in on-chip).
  Softmax: a bias tile B0[e,s] = (seg(e)!=s) * -3e4 (one fused two-op
  tensor_scalar on DVE) is matmul-accumulated into the score PSUM, so one
  batched exp(S+B0) on the Activation engine directly yields the masked
  softmax weights, which feed the ft/den matmuls as lhsT. Max-subtraction
  is skipped (|score| small, exact in f32). Per-window softmax denominators
  accumulate as a second PSUM group AFTER the ft group (PSUM accumulation
  groups must never interleave within a bank on real hardware).
- Stage 2 (item->target): masked-matmul pattern with bf16 operands; ft rows
  come via normal + transpose dma_gathers of the stage-1 output table. e2 is
  computed in transposed orientation (lhsT = qw, rhs = transposed ft/hp), so
  the per-edge weight w = <e2, f[dst]> becomes a row-select of W = e2T^T fT,
  done by one fused scalar_tensor_tensor with accum_out; no f-table DRAM
  round-trip or gather. Degree normalization (1/max(deg,1)) is host graph
  metadata. Stage-2 windows are emitted in small chunks interleaved between
  stage-1 batches as soon as their ft rows exist (cuts), hiding the tail.
- Numeric tables (h_v/h_p/h_t) are staged in bfloat16; all arithmetic
  (pi scaling, matmuls, softmax, tanh, means) runs on the NeuronCores with
  f32 PSUM accumulation. Host work is index math, row permutation/layout
  of input tables, and dtype staging.

kernel(**inputs) accepts the FULL unsharded inputs and returns the FULL
[N_TGT, 128] float32 output.
"""
import numpy as np
import ml_dtypes

BF16 = np.dtype(ml_dtypes.bfloat16)

P = 128          # partitions / tile edge
D = 128          # feature dim
NCORES = 8
WE1 = 1024       # stage-1 window edge capacity (8 tiles)
WS1 = 128        # stage-1 window slot capacity
WB = 8           # stage-1 windows per batch
RB = WB * WE1    # edge ranks per batch (8192)
TI2 = 50         # stage-2 tiles per window (6400 item slots)
WS2 = 128        # stage-2 window target capacity
GH = 4096        # gather granularity (half batch)
DEBUG_FT = False # expose stage-1 ft table as an output
_LAST_NC = None


def _wrap_idx16(idx: np.ndarray, cap: int) -> np.ndarray:
    """[n<=cap] -> [128, cap/16] int16 (j at [j%16, j//16], replicated x8)."""
    a = np.zeros(cap, np.int64)
    a[: idx.shape[0]] = idx
    assert cap % 16 == 0
    assert a.min() >= 0 and a.max() < 32768, (a.min(), a.max())
    blk = a.reshape(cap // 16, 16).T.astype(np.int16)
    return np.tile(blk, (8, 1))


def _interleave_f32(vals: np.ndarray, cap: int, fill: float) -> np.ndarray:
    """[n] -> [128, cap/128] f32 with value of rank r at [r%128, r//128]."""
    a = np.full(cap, fill, np.float32)
    a[: vals.shape[0]] = vals
    return a.reshape(cap // P, P).T.copy()


def _pack_runs(run_sizes, max_runs, max_total):
    """Greedy pack consecutive runs into groups of whole runs, <=max_runs
    runs and <=max_total total size. Returns list of (start_run, n_runs)."""
    groups = []
    i, n = 0, len(run_sizes)
    while i < n:
        tot, j = 0, i
        while j < n and j - i < max_runs and tot + run_sizes[j] <= max_total:
            tot += run_sizes[j]
            j += 1
        assert j > i, f"run {i} of size {run_sizes[i]} exceeds {max_total}"
        groups.append((i, j - i))
        i = j
    return groups


def preprocess(h_v, h_p, h_t, int_src, int_dst, agg_dst):
    """All graph restructuring. Returns shared dims + per-core arrays."""
    NITEM = h_v.shape[0]
    NTGT = h_t.shape[0]
    int_src = int_src.astype(np.int64)
    int_dst = int_dst.astype(np.int64)
    item_tgt = agg_dst.astype(np.int64)       # item i -> target (agg_src=arange)
    h_v_bf = h_v.astype(BF16)
    h_p_bf = h_p.astype(BF16)
    h_t_bf = h_t.astype(BF16)

    # ---- target -> core, balanced by stage-1 edge load ----
    deg_int = np.bincount(int_dst, minlength=NITEM)
    t_edges = np.bincount(item_tgt, weights=deg_int.astype(np.float64),
                          minlength=NTGT)
    t_items = np.bincount(item_tgt, minlength=NTGT)
    tgt_core = np.zeros(NTGT, np.int64)
    load = np.zeros(NCORES)
    for t in np.argsort(-t_edges, kind="stable"):
        c = int(np.argmin(load))
        tgt_core[t] = c
        load[c] += t_edges[t] + 0.5 * t_items[t]
    item_core = tgt_core[item_tgt]

    cores = []
    for c in range(NCORES):
        tlist = np.where(tgt_core == c)[0]
        items = np.where(item_core == c)[0]
        items = items[np.lexsort((items, item_tgt[items]))]
        cores.append({"targets": tlist, "items": items})

    # ---- stage-2 windows (whole targets, <=WS2 targets, <=TI2*128 islots) ----
    for c in range(NCORES):
        st = cores[c]
        st["w2groups"] = _pack_runs(t_items[st["targets"]], WS2, TI2 * P)
    W2 = max(len(st["w2groups"]) for st in cores)
    NI = W2 * TI2 * P

    for c in range(NCORES):
        st = cores[c]
        tl, items = st["targets"], st["items"]
        it_item = np.full(NI, -1, np.int64)        # islot -> global item
        it_tgtloc = np.full(NI, -1.0, np.float32)  # islot -> window-local tgt
        it_tslot = np.zeros(NI, np.int64)          # islot -> global tgt slot
        twin = np.full((W2, WS2), -1, np.int64)    # window -> global targets
        ipos = 0
        for w2, (t0, ntgt) in enumerate(st["w2groups"]):
            base = w2 * TI2 * P
            off = 0
            for k in range(ntgt):
                t = tl[t0 + k]
                cnt = int(t_items[t])
                sl = slice(base + off, base + off + cnt)
                it_item[sl] = items[ipos: ipos + cnt]
                it_tgtloc[sl] = k
                it_tslot[sl] = w2 * WS2 + k
                twin[w2, k] = t
                ipos += cnt
                off += cnt
        assert ipos == len(items)
        st["it_item"] = it_item
        st["it_tgtloc"] = it_tgtloc
        st["it_tslot"] = it_tslot
        st["twin"] = twin
        islot_of = np.full(NITEM, -1, np.int64)
        real = it_item >= 0
        islot_of[it_item[real]] = np.where(real)[0]
        st["islot_of"] = islot_of

    # ---- stage-1 windows: consecutive islot ranges ----
    for c in range(NCORES):
        st = cores[c]
        emask = item_core[int_dst] == c
        es = int_src[emask]
        ed = st["islot_of"][int_dst[emask]]
        o = np.argsort(ed, kind="stable")
        st["e_src"], st["e_dst"] = es[o], ed[o]
        cnt = np.bincount(st["e_dst"], minlength=NI)
        st["w1groups"] = _pack_runs(cnt, WS1, WE1)   # (islot0, nislots)
        st["islot_cnt"] = cnt
    W1 = max(len(st["w1groups"]) for st in cores)
    W1 = ((W1 + WB - 1) // WB) * WB
    B1 = W1 // WB
    assert W1 * WS1 <= 32768, f"ft table too big for int16: W1={W1}"

    for c in range(NCORES):
        st = cores[c]
        es, ed, cnt = st["e_src"], st["e_dst"], st["islot_cnt"]
        estart = np.concatenate([[0], np.cumsum(cnt)])
        wsrc = np.zeros((W1, WE1), np.int64)
        wseg = np.full((W1, WE1), -1.0, np.float32)
        ft_slot = np.zeros(NI, np.int64)
        wbase = np.full(W1, NI, np.int64)            # pad windows -> zero cols
        for w, (i0, ni) in enumerate(st["w1groups"]):
            e0, e1 = estart[i0], estart[i0 + ni]
            ne = int(e1 - e0)
            assert ne <= WE1 and ni <= WS1
            wsrc[w, :ne] = es[e0:e1]
            wseg[w, :ne] = (ed[e0:e1] - i0).astype(np.float32)
            ft_slot[i0: i0 + ni] = w * WS1 + np.arange(ni)
            wbase[w] = i0
        st["wsrc"], st["wseg"] = wsrc, wseg
        st["ft_slot"] = ft_slot
        st["wbase"] = wbase

    # ---- per-batch gather regions + index/seg arrays ----
    for c in range(NCORES):
        st = cores[c]
        hv2 = np.zeros((B1 * RB, D), BF16)
        g2 = np.zeros((B1, P, RB // 16), np.int16)
        seg = np.full((B1, P, RB // P), -1.0, np.float32)
        for b in range(B1):
            wins = slice(b * WB, (b + 1) * WB)
            src = st["wsrc"][wins].reshape(-1)
            sg = st["wseg"][wins].reshape(-1)
            real = sg >= 0
            uniq = np.unique(src[real])
            if uniq.size == 0:
                uniq = np.array([0], np.int64)
            assert uniq.size <= RB
            hv2[b * RB: b * RB + uniq.size] = h_v_bf[uniq]
            pos = np.zeros(RB, np.int64)
            pos[real] = np.searchsorted(uniq, src[real])
            g2[b] = _wrap_idx16(pos, RB)
            seg[b] = _interleave_f32(sg, RB, -1.0)
        st["hv2"], st["g2"], st["seg"] = hv2, g2, seg

        # window-padded dst table [D, W1*128] (col w*128+s = h_v[islot base+s])
        colitem = np.full(W1 * WS1, -1, np.int64)
        for w, (i0, ni) in enumerate(st["w1groups"]):
            colitem[w * WS1: w * WS1 + ni] = st["it_item"][i0: i0 + ni]
        hvlTw = np.zeros((D, W1 * WS1), BF16)
        cr = colitem >= 0
        hvlTw[:, cr] = h_v_bf[colitem[cr]].T
        st["hvlTw"] = hvlTw

    # ---- stage-2 gather/meta arrays + tables ----
    for c in range(NCORES):
        st = cores[c]
        it_item = st["it_item"]
        real = it_item >= 0
        st["ftg"] = _wrap_idx16(st["ft_slot"], NI)
        st["fexp"] = _wrap_idx16(st["it_tslot"], NI)
        tl = np.zeros((W2, P, TI2), np.float32)
        for w2 in range(W2):
            tl[w2] = _interleave_f32(
                st["it_tgtloc"][w2 * TI2 * P: (w2 + 1) * TI2 * P], TI2 * P,
                -1.0)
        st["tgtloc"] = tl
        # host-side degree normalization: deg[t] is graph structure
        r2 = np.ones((W2, P, 1), np.float32)
        tw2 = st["twin"]
        for w2 in range(W2):
            sel = tw2[w2] >= 0
            r2[w2, sel, 0] = 1.0 / np.maximum(t_items[tw2[w2][sel]], 1)
        st["rec2"] = r2
        hpT = np.zeros((D, NI), BF16)
        hpT[:, real] = h_p_bf[it_item[real]].T
        st["hpT"] = hpT
        htT = np.zeros((D, W2 * WS2), BF16)
        tw = st["twin"].reshape(-1)
        htT[:, tw >= 0] = h_t_bf[tw[tw >= 0]].T
        st["htT"] = htT

    # earliest stage-1 batch after which each stage-2 window's ft rows exist
    cuts = []
    for w2 in range(W2):
        E = (w2 + 1) * TI2 * P
        c_max = 0
        for c in range(NCORES):
            lastw = max(w for w, (i0, ni) in enumerate(cores[c]["w1groups"])
                        if i0 < E)
            c_max = max(c_max, lastw // WB)
        cuts.append(c_max)
    cuts = [max(cuts[: i + 1]) for i in range(W2)]
    cuts[W2 - 1] = B1 - 1

    dims = {"NI": NI, "W1": W1, "B1": B1, "W2": W2, "cuts": cuts,
            "NITEM": NITEM, "NTGT": NTGT}
    return dims, cores


# ======================= device program =======================

def build_program(dims):
    import concourse.bacc as bacc
    import concourse.mybir as mybir
    import concourse.tile as tile

    f32 = mybir.dt.float32
    bf16 = mybir.dt.bfloat16
    i16 = mybir.dt.int16
    Alu = mybir.AluOpType
    Act = mybir.ActivationFunctionType
    Ax = mybir.AxisListType

    NI, W1, B1, W2 = (dims[k] for k in ("NI", "W1", "B1", "W2"))
    FTC = W1 * WS1                     # dst-table columns / ft rows
    NW = TI2 * P                       # islots per stage-2 window

    nc = bacc.Bacc("TRN2", target_bir_lowering=False, debug=False,
                   num_devices=NCORES)
    # inputs
    hv2 = nc.dram_tensor("hv2", [B1 * RB, D], bf16, kind="ExternalInput")
    hvlTw = nc.dram_tensor("hvlTw", [D, FTC], bf16, kind="ExternalInput")
    hpT = nc.dram_tensor("hpT", [D, NI], bf16, kind="ExternalInput")
    htT = nc.dram_tensor("htT", [D, W2 * WS2], bf16, kind="ExternalInput")
    qw = nc.dram_tensor("qw", [2 * D, D], f32, kind="ExternalInput")
    rw = nc.dram_tensor("rw", [2 * D, D], f32, kind="ExternalInput")
    pic = nc.dram_tensor("pic", [D, 1], f32, kind="ExternalInput")
    iotab = nc.dram_tensor("iotab", [P, P], bf16, kind="ExternalInput")
    ident = nc.dram_tensor("ident", [P, P], f32, kind="ExternalInput")
    g2d = nc.dram_tensor("g2d", [B1, P, RB // 16], i16, kind="ExternalInput")
    segd = nc.dram_tensor("segd", [B1, P, RB // P], f32, kind="ExternalInput")
    ftgd = nc.dram_tensor("ftgd", [P, NI // 16], i16, kind="ExternalInput")
    tgtlocd = nc.dram_tensor("tgtlocd", [W2, P, TI2], f32, kind="ExternalInput")
    rec2d = nc.dram_tensor("rec2d", [W2, P, 1], f32, kind="ExternalInput")
    # output
    outd = nc.dram_tensor("out", [W2 * WS2, D], f32, kind="ExternalOutput")
    # internal scratch
    ftd = nc.dram_tensor("ft", [FTC, D], bf16,
                         kind="ExternalOutput" if DEBUG_FT else "Internal")

    with tile.TileContext(nc) as tc:
        with (
            tc.tile_pool(name="consts", bufs=1) as cp,
            tc.tile_pool(name="weights", bufs=1) as wp,
        ):
            iota_t = cp.tile([P, P], bf16)
            nc.sync.dma_start(out=iota_t[:], in_=iotab[:])
            ident_t = cp.tile([P, P], f32)
            nc.sync.dma_start(out=ident_t[:], in_=ident[:])
            ident_b = cp.tile([P, P], bf16)
            nc.scalar.activation(out=ident_b[:], in_=ident_t[:], func=Act.Copy)
            ones_b = cp.tile([P, 1], bf16)
            nc.vector.memset(ones_b[:], 1.0)
            pi_t = cp.tile([D, 1], f32)
            nc.sync.dma_start(out=pi_t[:], in_=pic[:])
            # weights: load f32, cast to bf16 on device
            qwf = wp.tile([P, 2, D], f32)
            nc.sync.dma_start(out=qwf[:, 0, :], in_=qw[0:D, :])
            nc.sync.dma_start(out=qwf[:, 1, :], in_=qw[D: 2 * D, :])
            qwb_t = wp.tile([P, 2, D], bf16)
            nc.scalar.activation(out=qwb_t[:], in_=qwf[:], func=Act.Copy)
            rwf = wp.tile([P, 2, D], f32)
            nc.sync.dma_start(out=rwf[:, 0, :], in_=rw[0:D, :])
            nc.sync.dma_start(out=rwf[:, 1, :], in_=rw[D: 2 * D, :])
            rwb_t = wp.tile([P, 2, D], bf16)
            nc.scalar.activation(out=rwb_t[:], in_=rwf[:], func=Act.Copy)

            # ---- P1 batches with stage-2 windows interleaved at cuts ----
            cuts = dims["cuts"]
            from contextlib import ExitStack
            with ExitStack() as stack:
                pool = lambda *a, **k: stack.enter_context(
                    tc.tile_pool(*a, **k))
                ip1 = pool(name="idx1", bufs=3)
                gp = pool(name="gat", bufs=4)
                tp = pool(name="xsT1", bufs=10)
                xp = pool(name="ex1", bufs=8)
                mp = pool(name="mx1", bufs=14)
                sm = pool(name="sm1", bufs=12)
                fsp = pool(name="fts", bufs=3)
                ip2 = pool(name="idx2", bufs=1)
                bg = pool(name="big2", bufs=1)
                wk2 = pool(name="wk2", bufs=10)
                xp2 = pool(name="ex2", bufs=10)
                sm2 = pool(name="sm2", bufs=8)
                psS = pool(name="psS", bufs=2, space="PSUM")
                psT = pool(name="psT", bufs=2, space="PSUM")
                psF = pool(name="psF", bufs=2, space="PSUM")
                ppA = pool(name="psA", bufs=1, space="PSUM")
                ppB = pool(name="psB", bufs=1, space="PSUM")
                ftgt = ip2.tile([P, NI // 16], i16, tag="ftg")
                nc.sync.dma_start(out=ftgt[:], in_=ftgd[:])

                def s1_batch(b):
                    g2t = ip1.tile([P, RB // 16], i16, tag="g2")
                    nc.sync.dma_start(out=g2t[:], in_=g2d[b])
                    segt = ip1.tile([P, RB // P], f32, tag="seg")
                    nc.sync.dma_start(out=segt[:], in_=segd[b])
                    xdw0 = ip1.tile([P, WB * WS1], bf16, tag="xdw0")
                    nc.sync.dma_start(
                        out=xdw0[:],
                        in_=hvlTw[:, b * WB * WS1: (b + 1) * WB * WS1])
                    xdw = ip1.tile([P, WB * WS1], bf16, tag="xdw")
                    nc.vector.tensor_scalar_mul(xdw[:], xdw0[:], pi_t[:])
                    ftstage = fsp.tile([P, WB, D], bf16, tag="fts")
                    for h in range(2):
                        xs = gp.tile([P, GH // P, D], bf16, tag="xs")
                        nc.gpsimd.dma_gather(
                            out_ap=xs[:], in_ap=hv2[b * RB: (b + 1) * RB],
                            idxs_ap=g2t[:, h * GH // 16: (h + 1) * GH // 16],
                            num_idxs=GH, num_idxs_reg=GH, elem_size=D,
                            single_packet=False)
                        for wl in range(WB // 2):
                            w = h * (WB // 2) + wl
                            fdp = psF.tile([P, D + 1], f32,
                                           space="PSUM", tag="ftden")
                            ftp = fdp[:, 0:D]
                            denp = fdp[:, D: D + 1]
                            exg = []
                            for g in range(2):
                                # transpose 4 tiles: [e,d] -> [d,e]
                                trp = psT.tile([P, 4 * P], bf16, space="PSUM",
                                               tag="tr")
                                for j in range(4):
                                    t = wl * 8 + g * 4 + j   # tile in half
                                    nc.tensor.transpose(
                                        out=trp[:, j * P: (j + 1) * P],
                                        in_=xs[:, t, :], identity=ident_b[:])
                                xsT = tp.tile([P, 4 * P], bf16, tag="xsT")
                                if (wl * 2 + g) % 2 == 0:
                                    nc.scalar.activation(out=xsT[:],
                                                         in_=trp[:],
                                                         func=Act.Copy)
                                else:
                                    nc.vector.tensor_copy(out=xsT[:],
                                                          in_=trp[:])
                                sp = psS.tile([P, 4 * P], f32, space="PSUM",
                                              tag="sp")
                                for j in range(4):
                                    tb = w * 8 + g * 4 + j   # tile in batch
                                    # B0 = (iota != seg) * -30000: exp(S+B0)
                                    # is the masked softmax weight directly
                                    b0 = mp.tile([P, P], bf16, tag="b0")
                                    nc.vector.tensor_scalar(
                                        out=b0[:], in0=iota_t[:],
                                        scalar1=segt[:, tb: tb + 1],
                                        scalar2=-30000.0,
                                        op0=Alu.not_equal, op1=Alu.mult)
                                    nc.tensor.matmul(
                                        out=sp[:, j * P: (j + 1) * P],
                                        lhsT=xsT[:, j * P: (j + 1) * P],
                                        rhs=xdw[:, w * WS1: (w + 1) * WS1],
                                        start=True, stop=False)
                                    nc.tensor.matmul(
                                        out=sp[:, j * P: (j + 1) * P],
                                        lhsT=ident_b[:], rhs=b0[:],
                                        start=False, stop=True)
                                ex = xp.tile([P, 4 * P], bf16, tag="ex")
                                nc.scalar.activation(out=ex[:], in_=sp[:],
                                                     func=Act.Exp)
                                exg.append(ex)
                                for j in range(4):
                                    t = wl * 8 + g * 4 + j
                                    i = g * 4 + j            # tile in window
                                    nc.tensor.matmul(
                                        out=ftp,
                                        lhsT=ex[:, j * P: (j + 1) * P],
                                        rhs=xs[:, t, :],
                                        start=(i == 0), stop=(i == 7))
                            # den group AFTER the ft group: accumulation
                            # groups must not interleave within a PSUM bank
                            for i in range(8):
                                nc.tensor.matmul(
                                    out=denp,
                                    lhsT=exg[i // 4][:, (i % 4) * P:
                                                     (i % 4 + 1) * P],
                                    rhs=ones_b[:],
                                    start=(i == 0), stop=(i == 7))
                            denc = sm.tile([P, 1], f32, tag="denc")
                            nc.vector.tensor_scalar_max(denc[:], denp,
                                                        1e-30)
                            rec = sm.tile([P, 1], f32, tag="rec")
                            nc.vector.reciprocal(rec[:], denc[:])
                            nc.vector.tensor_scalar_mul(
                                ftstage[:, w, :], ftp, rec[:])
                    nc.scalar.dma_start(
                        out=ftd[b * WB * WS1: (b + 1) * WB * WS1, :].rearrange(
                            "(w p) d -> p w d", p=P),
                        in_=ftstage[:])

                def s2_c0(w2, st2):
                    hpt = bg.tile([P, NW], bf16, tag="hpt")
                    nc.sync.dma_start(out=hpt[:],
                                      in_=hpT[:, w2 * NW: (w2 + 1) * NW])
                    tlt = sm2.tile([P, TI2], f32, tag="tlt")
                    nc.sync.dma_start(out=tlt[:], in_=tgtlocd[w2])
                    ftg = bg.tile([P, TI2, D], bf16, tag="ftgw")
                    ftgT = bg.tile([P, 1, NW], bf16, tag="ftgTw")
                    for o0, n in ((0, 4096), (4096, NW - 4096)):
                        o = w2 * NW + o0
                        nc.gpsimd.dma_gather(
                            out_ap=ftg[:, o0 // P: (o0 + n) // P, :],
                            in_ap=ftd[:],
                            idxs_ap=ftgt[:, o // 16: (o + n) // 16],
                            num_idxs=n, num_idxs_reg=n, elem_size=D,
                            single_packet=False)
                        nc.gpsimd.dma_gather(
                            out_ap=ftgT[:, :, o0: o0 + n],
                            in_ap=ftd[:],
                            idxs_ap=ftgt[:, o // 16: (o + n) // 16],
                            num_idxs=n, num_idxs_reg=n, elem_size=D,
                            single_packet=False, transpose=True)
                    st2.update(hpt=hpt, tlt=tlt, ftg=ftg, ftgT=ftgT)

                def s2_c1(w2, st2):
                    hpt, tlt, ftg, ftgT = (st2[k] for k in
                                           ("hpt", "tlt", "ftg", "ftgT"))
                    # one PSUM bank per window: mean | f | out, with
                    # strictly sequential accumulation groups (groups must
                    # not interleave within a PSUM bank)
                    apo = ppA.tile([P, 3 * D], f32, space="PSUM", tag="apo")
                    meanp = apo[:, 0:D]
                    fp = apo[:, D: 2 * D]
                    outp = apo[:, 2 * D: 3 * D]
                    rec2 = sm2.tile([P, 1], f32, tag="rec2")
                    nc.sync.dma_start(out=rec2[:], in_=rec2d[w2])
                    st2.update(apo=apo, meanp=meanp, fp=fp, outp=outp,
                               rec2=rec2)
                    # sweep A: mean (first half); deg comes from the host
                    for i in range(TI2 // 2):
                        mask = wk2.tile([P, P], bf16, tag="maskA")
                        nc.vector.tensor_scalar(
                            out=mask[:], in0=iota_t[:],
                            scalar1=tlt[:, i: i + 1], scalar2=None,
                            op0=Alu.is_equal)
                        nc.tensor.matmul(out=meanp, lhsT=mask[:],
                                         rhs=ftg[:, i, :],
                                         start=(i == 0), stop=(i == TI2 - 1))
                def s2_c2(w2, st2):
                    hpt, tlt, ftg, ftgT = (st2[k] for k in
                                           ("hpt", "tlt", "ftg", "ftgT"))
                    meanp, fp, rec2 = (st2[k] for k in
                                       ("meanp", "fp", "rec2"))
                    for i in range(TI2 // 2, TI2):
                        mask = wk2.tile([P, P], bf16, tag="maskA")
                        nc.vector.tensor_scalar(
                            out=mask[:], in0=iota_t[:],
                            scalar1=tlt[:, i: i + 1], scalar2=None,
                            op0=Alu.is_equal)
                        nc.tensor.matmul(out=meanp, lhsT=mask[:],
                                         rhs=ftg[:, i, :],
                                         start=(i == 0), stop=(i == TI2 - 1))
                    mean_sb = wk2.tile([P, D], f32, tag="mean_sb")
                    nc.vector.tensor_scalar_mul(mean_sb[:], meanp, rec2[:])
                    trx = ppB.tile([P, 4 * P], f32, space="PSUM", tag="big")
                    nc.tensor.transpose(out=trx[:, 0:P], in_=mean_sb[:],
                                        identity=ident_t[:])
                    meanT = wk2.tile([P, P], bf16, tag="meanT")
                    nc.scalar.activation(out=meanT[:], in_=trx[:, 0:P],
                                         func=Act.Copy)
                    htt = wk2.tile([P, P], bf16, tag="htt")
                    nc.sync.dma_start(out=htt[:],
                                      in_=htT[:, w2 * WS2: (w2 + 1) * WS2])
                    nc.tensor.matmul(out=fp, lhsT=htt[:], rhs=rwb_t[:, 0, :],
                                     start=True, stop=False)
                    nc.tensor.matmul(out=fp, lhsT=meanT[:],
                                     rhs=rwb_t[:, 1, :],
                                     start=False, stop=True)
                    # fT = f transposed [d, tgt] (for W = e2T^T @ fT)
                    f_sb = wk2.tile([P, D], f32, tag="f_sb")
                    nc.vector.tensor_copy(out=f_sb[:], in_=fp)
                    trf = ppB.tile([P, 4 * P], f32, space="PSUM", tag="big")
                    nc.tensor.transpose(out=trf[:, 0:P], in_=f_sb[:],
                                        identity=ident_t[:])
                    fTb = wk2.tile([P, P], bf16, tag="fTb")
                    nc.scalar.activation(out=fTb[:], in_=trf[:, 0:P],
                                         func=Act.Copy)
                    st2.update(fTb=fTb)

                def s2_swb(w2, st2, g0s):
                    hpt, tlt, ftg, ftgT, fTb, outp = (st2[k] for k in
                        ("hpt", "tlt", "ftg", "ftgT", "fTb", "outp"))
                    # sweep B: e2T = tanh(qw^T [ft, hp]^T); W = e2T^T fT;
                    # wc[islot] = W[islot, tl(islot)] via fused mask+reduce
                    for g0 in g0s:
                        gn = min(4, TI2 - g0)
                        e2p = ppB.tile([P, 4 * P], f32, space="PSUM",
                                       tag="big")
                        for j in range(gn):
                            i = g0 + j
                            nc.tensor.matmul(
                                out=e2p[:, j * P: (j + 1) * P],
                                lhsT=qwb_t[:, 0, :],
                                rhs=ftgT[:, 0, i * P: (i + 1) * P],
                                start=True, stop=False)
                            nc.tensor.matmul(
                                out=e2p[:, j * P: (j + 1) * P],
                                lhsT=qwb_t[:, 1, :],
                                rhs=hpt[:, i * P: (i + 1) * P],
                                start=False, stop=True)
                        e2T = xp2.tile([P, 4 * P], bf16, tag="e2sb")
                        nc.scalar.activation(out=e2T[:, : gn * P],
                                             in_=e2p[:, : gn * P],
                                             func=Act.Tanh)
                        wp_ = ppB.tile([P, 4 * P], f32, space="PSUM",
                                       tag="big")
                        for j in range(gn):
                            nc.tensor.matmul(
                                out=wp_[:, j * P: (j + 1) * P],
                                lhsT=e2T[:, j * P: (j + 1) * P],
                                rhs=fTb[:], start=True, stop=True)
                        for j in range(gn):
                            i = g0 + j
                            wsel = xp2.tile([P, P], bf16, tag="wsel")
                            wc = sm2.tile([P, 1], f32, tag="wc")
                            nc.vector.scalar_tensor_tensor(
                                out=wsel[:], in0=iota_t[:],
                                scalar=tlt[:, i: i + 1],
                                in1=wp_[:, j * P: (j + 1) * P],
                                op0=Alu.is_equal, op1=Alu.mult,
                                accum_out=wc[:])
                            maskw = wk2.tile([P, P], bf16, tag="maskw")
                            nc.vector.tensor_scalar(
                                out=maskw[:], in0=iota_t[:],
                                scalar1=tlt[:, i: i + 1],
                                scalar2=wc[:],
                                op0=Alu.is_equal, op1=Alu.mult)
                            nc.tensor.matmul(out=outp, lhsT=maskw[:],
                                             rhs=ftg[:, i, :],
                                             start=(i == 0),
                                             stop=(i == TI2 - 1))
                def s2_c3(w2, st2):
                    s2_swb(w2, st2, range(0, 24, 4))

                def s2_c4(w2, st2):
                    s2_swb(w2, st2, range(24, TI2, 4))
                    outp = st2["outp"]
                    out_sb = wk2.tile([P, D], f32, tag="out_sb")
                    nc.vector.tensor_copy(out=out_sb[:], in_=outp)
                    nc.scalar.dma_start(out=outd[w2 * WS2: (w2 + 1) * WS2, :],
                                        in_=out_sb[:])

                chunks = [s2_c0, s2_c1, s2_c2, s2_c3, s2_c4]
                sched = {}          # batch -> list of (chunk_fn, w2)
                states = [dict() for _ in range(W2)]
                for w2 in range(W2):
                    for ci, fn in enumerate(chunks):
                        sched.setdefault(cuts[w2] + ci, []).append((fn, w2))
                for b in range(B1):
                    s1_batch(b)
                    for fn, w2 in sched.get(b, []):
                        fn(w2, states[w2])
                for b in range(B1, max(sched) + 1):
                    for fn, w2 in sched.get(b, []):
                        fn(w2, states[w2])
    nc.compile()
    return nc


def make_in_maps(dims, cores, pi_w, q_w, r_w):
    iota_bf = np.tile(np.arange(P, dtype=np.float32), (P, 1)).astype(BF16)
    ident = np.eye(P, dtype=np.float32)
    in_maps = []
    for c in range(NCORES):
        st = cores[c]
        in_maps.append({
            "hv2": st["hv2"],
            "hvlTw": st["hvlTw"],
            "hpT": st["hpT"],
            "htT": st["htT"],
            "qw": np.ascontiguousarray(q_w, np.float32),
            "rw": np.ascontiguousarray(r_w, np.float32),
            "pic": np.ascontiguousarray(pi_w.reshape(D, 1), np.float32),
            "iotab": iota_bf, "ident": ident,
            "g2d": st["g2"], "segd": st["seg"],
            "ftgd": st["ftg"],
            "tgtlocd": st["tgtloc"], "rec2d": st["rec2"],
        })
    return in_maps


def unshard(dims, cores, results):
    NTGT = dims["NTGT"]
    out = np.zeros((NTGT, D), np.float32)
    for c in range(NCORES):
        st = cores[c]
        o = results[c]["out"]
        tw = st["twin"]
        for w2 in range(dims["W2"]):
            sel = tw[w2] >= 0
            out[tw[w2][sel]] = o[w2 * WS2: w2 * WS2 + WS2][sel]
    return out


def kernel(**inputs):
    from concourse.bass_utils import run_bass_kernel_spmd

    h_v = np.asarray(inputs["h_v"], np.float32)
    h_p = np.asarray(inputs["h_p"], np.float32)
    h_t = np.asarray(inputs["h_t"], np.float32)
    pi_w = np.asarray(inputs["pi_w"], np.float32)
    q_w = np.asarray(inputs["q_w"], np.float32)
    r_w = np.asarray(inputs["r_w"], np.float32)
    int_src = np.asarray(inputs["int_src"]).astype(np.int64)
    int_dst = np.asarray(inputs["int_dst"]).astype(np.int64)
    agg_src = np.asarray(inputs["agg_src"]).astype(np.int64)
    agg_dst = np.asarray(inputs["agg_dst"]).astype(np.int64)
    assert np.array_equal(agg_src, np.arange(agg_src.shape[0])), \
        "kernel assumes agg_src == arange (per problem spec fill)"

    dims, cores = preprocess(h_v, h_p, h_t, int_src, int_dst, agg_dst)
    nc = build_program(dims)
    global _LAST_NC
    _LAST_NC = nc
    in_maps = make_in_maps(dims, cores, pi_w, q_w, r_w)
    res = run_bass_kernel_spmd(nc, in_maps, core_ids=list(range(NCORES)))
    return unshard(dims, cores, res.results)



# revision 4
# speedup vs baseline: 1.2708x; 1.2708x over previous
"""Trainium2 Bass kernel for nn_DglAggregator (GNN message passing).

Strategy (8 NeuronCores, SPMD, one uniform program, per-core data):
- Targets are partitioned across cores balanced by stage-1 edge count; each
  core owns its targets' items and ALL stage-1 edges pointing at those items,
  so no cross-core communication is needed.
- Stage 1 (item->item segment softmax + weighted sum): items are packed into
  "units" of <=32 whole slots and <=256 edges (2 edge tiles of 128); 4 units
  form a "block" (<=128 slots), 8 blocks form a batch (8192 edge ranks).
  The per-edge source rows arrive as ONE linear 2MB DMA per batch from a
  host-relaid edge-rank-ordered table (with a 129th all-ones column); no
  device gathers.  Narrow score tiles: S[e, 0:32] is computed against the
  unit's 32 dst columns only (h_v[dst]*pi, streamed compactly), the segment
  mask arrives as a host-streamed bias table b0 (0 / -30000) added via a
  second accumulating matmul, so one batched exp per block yields masked
  softmax weights directly.  The ft matmul contracts edges with rhs
  [e,129] = [src_feat | 1], producing ft AND the softmax denominator in one
  accumulation group per unit at its partition offset; normalization happens
  in the PSUM->SBUF copy (per-partition reciprocal scale).  Max-subtraction
  is skipped (|score| small, exact in f32).
- Stage 2 (item->target): masked-matmul pattern with bf16 operands; ft rows
  come via normal + transpose dma_gathers of the stage-1 output table. e2 is
  computed in transposed orientation (lhsT = qw, rhs = transposed ft/hp), so
  the per-edge weight w = <e2, f[dst]> becomes a row-select of W = e2T^T fT,
  done by one fused scalar_tensor_tensor with accum_out.  Degree
  normalization (1/max(deg,1)) is host graph metadata.  Stage-2 windows are
  emitted in small chunks interleaved between stage-1 batches as soon as
  their ft rows exist (cuts), hiding the tail.
- Numeric tables (h_v/h_p/h_t) are staged in bfloat16; all arithmetic
  (pi scaling, matmuls, softmax, tanh, means) runs on the NeuronCores with
  f32 PSUM accumulation. Host work is index math, row permutation/layout
  of input tables, and dtype staging.

kernel(**inputs) accepts the FULL unsharded inputs and returns the FULL
[N_TGT, 128] float32 output.
"""
import numpy as np
import ml_dtypes

BF16 = np.dtype(ml_dtypes.bfloat16)

P = 128          # partitions / tile edge
D = 128          # feature dim
NCORES = 8
SU = 32          # slots per unit
EU = 256         # edge capacity per unit (2 tiles)
UPB = 32         # units per batch
TPB = 2 * UPB    # edge tiles per batch (64)
BPB = UPB // 4   # blocks per batch (8); block = 4 units = 128 slots
RB = TPB * P     # edge ranks per batch (8192)
TI2 = 50         # stage-2 tiles per window (6400 item slots)
WS2 = 128        # stage-2 window target capacity
_LAST_NC = None


def _wrap_idx16(idx: np.ndarray, cap: int) -> np.ndarray:
    """[n<=cap] -> [128, cap/16] int16 (j at [j%16, j//16], replicated x8)."""
    a = np.zeros(cap, np.int64)
    a[: idx.shape[0]] = idx
    assert cap % 16 == 0
    assert a.min() >= 0 and a.max() < 32768, (a.min(), a.max())
    blk = a.reshape(cap // 16, 16).T.astype(np.int16)
    return np.tile(blk, (8, 1))


def _interleave_f32(vals: np.ndarray, cap: int, fill: float) -> np.ndarray:
    """[n] -> [128, cap/128] f32 with value of rank r at [r%128, r//128]."""
    a = np.full(cap, fill, np.float32)
    a[: vals.shape[0]] = vals
    return a.reshape(cap // P, P).T.copy()


def _pack_runs(run_sizes, max_runs, max_total):
    """Greedy pack consecutive runs into groups of whole runs, <=max_runs
    runs and <=max_total total size. Returns list of (start_run, n_runs)."""
    groups = []
    i, n = 0, len(run_sizes)
    while i < n:
        tot, j = 0, i
        while j < n and j - i < max_runs and tot + run_sizes[j] <= max_total:
            tot += run_sizes[j]
            j += 1
        assert j > i, f"run {i} of size {run_sizes[i]} exceeds {max_total}"
        groups.append((i, j - i))
        i = j
    return groups


def preprocess(h_v, h_p, h_t, int_src, int_dst, agg_dst):
    """All graph restructuring. Returns shared dims + per-core arrays."""
    NITEM = h_v.shape[0]
    NTGT = h_t.shape[0]
    int_src = int_src.astype(np.int64)
    int_dst = int_dst.astype(np.int64)
    item_tgt = agg_dst.astype(np.int64)       # item i -> target (agg_src=arange)
    h_v_bf = h_v.astype(BF16)
    h_p_bf = h_p.astype(BF16)
    h_t_bf = h_t.astype(BF16)

    # ---- target -> core, balanced by stage-1 edge load ----
    deg_int = np.bincount(int_dst, minlength=NITEM)
    t_edges = np.bincount(item_tgt, weights=deg_int.astype(np.float64),
                          minlength=NTGT)
    t_items = np.bincount(item_tgt, minlength=NTGT)
    tgt_core = np.zeros(NTGT, np.int64)
    load = np.zeros(NCORES)
    for t in np.argsort(-t_edges, kind="stable"):
        c = int(np.argmin(load))
        tgt_core[t] = c
        load[c] += t_edges[t] + 0.5 * t_items[t]
    item_core = tgt_core[item_tgt]

    cores = []
    for c in range(NCORES):
        tlist = np.where(tgt_core == c)[0]
        items = np.where(item_core == c)[0]
        items = items[np.lexsort((items, item_tgt[items]))]
        cores.append({"targets": tlist, "items": items})

    # ---- stage-2 windows (whole targets, <=WS2 targets, <=TI2*128 islots) ----
    for c in range(NCORES):
        st = cores[c]
        st["w2groups"] = _pack_runs(t_items[st["targets"]], WS2, TI2 * P)
    W2 = max(len(st["w2groups"]) for st in cores)
    NI = W2 * TI2 * P

    for c in range(NCORES):
        st = cores[c]
        tl, items = st["targets"], st["items"]
        it_item = np.full(NI, -1, np.int64)        # islot -> global item
        it_tgtloc = np.full(NI, -1.0, np.float32)  # islot -> window-local tgt
        it_tslot = np.zeros(NI, np.int64)          # islot -> global tgt slot
        twin = np.full((W2, WS2), -1, np.int64)    # window -> global targets
        ipos = 0
        for w2, (t0, ntgt) in enumerate(st["w2groups"]):
            base = w2 * TI2 * P
            off = 0
            for k in range(ntgt):
                t = tl[t0 + k]
                cnt = int(t_items[t])
                sl = slice(base + off, base + off + cnt)
                it_item[sl] = items[ipos: ipos + cnt]
                it_tgtloc[sl] = k
                it_tslot[sl] = w2 * WS2 + k
                twin[w2, k] = t
                ipos += cnt
                off += cnt
        assert ipos == len(items)
        st["it_item"] = it_item
        st["it_tgtloc"] = it_tgtloc
        st["it_tslot"] = it_tslot
        st["twin"] = twin

    # ---- stage-1 unit packing (whole items, <=SU slots, <=EU edges) ----
    for c in range(NCORES):
        st = cores[c]
        items = st["items"]
        npos = len(items)
        degs = deg_int[items]
        unit_id = np.zeros(npos, np.int64)
        local_slot = np.zeros(npos, np.int64)
        u, ecnt, scnt = 0, 0, 0
        for q in range(npos):
            d = int(degs[q])
            if ecnt + d > EU or scnt >= SU:
                u += 1
                ecnt, scnt = 0, 0
            unit_id[q] = u
            local_slot[q] = scnt
            ecnt += d
            scnt += 1
        st["unit_id"] = unit_id
        st["local_slot"] = local_slot
        st["n_units"] = (u + 1) if npos else 0
    B1 = max((st["n_units"] + UPB - 1) // UPB for st in cores)
    NU32 = B1 * UPB * SU                         # rows of the ft table
    assert NU32 < 32768, f"ft table too big for int16 gather: {NU32}"

    # ---- per-core edge-stream tables ----
    for c in range(NCORES):
        st = cores[c]
        items = st["items"]
        npos = len(items)
        unit_id, local_slot = st["unit_id"], st["local_slot"]
        pos_of = np.full(NITEM, -1, np.int64)
        pos_of[items] = np.arange(npos)

        emask = item_core[int_dst] == c
        es = int_src[emask]
        epos = pos_of[int_dst[emask]]
        o = np.argsort(epos, kind="stable")
        es, epos = es[o], epos[o]
        ne = es.shape[0]

        eu = unit_id[epos]                       # unit of each edge
        ustart = np.searchsorted(eu, np.arange(st["n_units"]))
        rank = np.arange(ne) - ustart[eu]        # rank within unit (<EU)
        assert rank.max(initial=0) < EU
        eb = eu // UPB                           # batch
        etile = (eu % UPB) * 2 + rank // P       # tile within batch
        ep = rank % P                            # partition row
        eseg = local_slot[epos]                  # unit-local slot

        xs = np.zeros((B1, P, TPB, D + 1), BF16)
        xs[eb, ep, etile, :D] = h_v_bf[es]
        xs[:, :, :, D] = 1.0
        b0 = np.full((B1, P, TPB * SU), -30000.0, BF16)
        b0[eb, ep, etile * SU + eseg] = 0.0
        st["xs"] = xs.reshape(B1, P, TPB * (D + 1))
        st["b0"] = b0

        # compact dst columns: unit u slot s -> h_v[item]
        xdw0 = np.zeros((B1, D, UPB * SU), BF16)
        qb = unit_id // UPB
        qcol = (unit_id % UPB) * SU + local_slot
        xdw0[qb, :, qcol] = h_v_bf[items]
        st["xdw0"] = xdw0

        # item -> ft table row (unified islot)
        uslot = unit_id * SU + local_slot
        it_item = st["it_item"]
        real = it_item >= 0
        ft_slot = np.zeros(NI, np.int64)
        ft_slot[real] = uslot[pos_of[it_item[real]]]
        st["ft_slot"] = ft_slot
        st["ftg"] = _wrap_idx16(ft_slot, NI)

    # ---- stage-2 gather/meta arrays + tables ----
    for c in range(NCORES):
        st = cores[c]
        it_item = st["it_item"]
        real = it_item >= 0
        tl = np.zeros((W2, P, TI2), np.float32)
        for w2 in range(W2):
            tl[w2] = _interleave_f32(
                st["it_tgtloc"][w2 * TI2 * P: (w2 + 1) * TI2 * P], TI2 * P,
                -1.0)
        st["tgtloc"] = tl
        # host-side degree normalization: deg[t] is graph structure
        r2 = np.ones((W2, P, 1), np.float32)
        tw2 = st["twin"]
        for w2 in range(W2):
            sel = tw2[w2] >= 0
            r2[w2, sel, 0] = 1.0 / np.maximum(t_items[tw2[w2][sel]], 1)
        st["rec2"] = r2
        hpT = np.zeros((D, NI), BF16)
        hpT[:, real] = h_p_bf[it_item[real]].T
        st["hpT"] = hpT
        htT = np.zeros((D, W2 * WS2), BF16)
        tw = st["twin"].reshape(-1)
        htT[:, tw >= 0] = h_t_bf[tw[tw >= 0]].T
        st["htT"] = htT

    # earliest stage-1 batch after which each stage-2 window's ft rows exist
    cuts = []
    for w2 in range(W2):
        E = (w2 + 1) * TI2 * P
        c_max = 0
        for c in range(NCORES):
            st = cores[c]
            sel = st["it_item"][:E] >= 0
            if sel.any():
                pos_of = np.full(NITEM, -1, np.int64)
                pos_of[st["items"]] = np.arange(len(st["items"]))
                lastu = int(st["unit_id"][
                    pos_of[st["it_item"][:E][sel]]].max())
                c_max = max(c_max, lastu // UPB)
        cuts.append(c_max)
    cuts = [max(cuts[: i + 1]) for i in range(W2)]
    cuts[W2 - 1] = B1 - 1

    dims = {"NI": NI, "B1": B1, "W2": W2, "NU32": NU32, "cuts": cuts,
            "NITEM": NITEM, "NTGT": NTGT}
    return dims, cores


# ======================= device program =======================

def build_program(dims):
    import concourse.bacc as bacc
    import concourse.mybir as mybir
    import concourse.tile as tile

    f32 = mybir.dt.float32
    bf16 = mybir.dt.bfloat16
    i16 = mybir.dt.int16
    Alu = mybir.AluOpType
    Act = mybir.ActivationFunctionType

    NI, B1, W2, NU32 = (dims[k] for k in ("NI", "B1", "W2", "NU32"))
    NW = TI2 * P                       # islots per stage-2 window
    GH = 4096

    nc = bacc.Bacc("TRN2", target_bir_lowering=False, debug=False,
                   num_devices=NCORES)
    # inputs
    xsd = nc.dram_tensor("xsd", [B1, P, TPB * (D + 1)], bf16,
                         kind="ExternalInput")
    xdw0d = nc.dram_tensor("xdw0d", [B1, D, UPB * SU], bf16,
                           kind="ExternalInput")
    b0d = nc.dram_tensor("b0d", [B1, P, TPB * SU], bf16,
                         kind="ExternalInput")
    hpT = nc.dram_tensor("hpT", [D, NI], bf16, kind="ExternalInput")
    htT = nc.dram_tensor("htT", [D, W2 * WS2], bf16, kind="ExternalInput")
    qw = nc.dram_tensor("qw", [2 * D, D], f32, kind="ExternalInput")
    rw = nc.dram_tensor("rw", [2 * D, D], f32, kind="ExternalInput")
    pic = nc.dram_tensor("pic", [D, 1], f32, kind="ExternalInput")
    iotab = nc.dram_tensor("iotab", [P, P], bf16, kind="ExternalInput")
    ident = nc.dram_tensor("ident", [P, P], f32, kind="ExternalInput")
    ftgd = nc.dram_tensor("ftgd", [P, NI // 16], i16, kind="ExternalInput")
    tgtlocd = nc.dram_tensor("tgtlocd", [W2, P, TI2], f32, kind="ExternalInput")
    rec2d = nc.dram_tensor("rec2d", [W2, P, 1], f32, kind="ExternalInput")
    # output
    outd = nc.dram_tensor("out", [W2 * WS2, D], f32, kind="ExternalOutput")
    # internal scratch
    ftd = nc.dram_tensor("ft", [NU32, D], bf16, kind="Internal")

    with tile.TileContext(nc) as tc:
        with (
            tc.tile_pool(name="consts", bufs=1) as cp,
            tc.tile_pool(name="weights", bufs=1) as wp,
        ):
            iota_t = cp.tile([P, P], bf16)
            nc.sync.dma_start(out=iota_t[:], in_=iotab[:])
            ident_t = cp.tile([P, P], f32)
            nc.sync.dma_start(out=ident_t[:], in_=ident[:])
            ident_b = cp.tile([P, P], bf16)
            nc.scalar.activation(out=ident_b[:], in_=ident_t[:], func=Act.Copy)
            pi_t = cp.tile([D, 1], f32)
            nc.sync.dma_start(out=pi_t[:], in_=pic[:])
            # weights: load f32, cast to bf16 on device
            qwf = wp.tile([P, 2, D], f32)
            nc.sync.dma_start(out=qwf[:, 0, :], in_=qw[0:D, :])
            nc.sync.dma_start(out=qwf[:, 1, :], in_=qw[D: 2 * D, :])
            qwb_t = wp.tile([P, 2, D], bf16)
            nc.scalar.activation(out=qwb_t[:], in_=qwf[:], func=Act.Copy)
            rwf = wp.tile([P, 2, D], f32)
            nc.sync.dma_start(out=rwf[:, 0, :], in_=rw[0:D, :])
            nc.sync.dma_start(out=rwf[:, 1, :], in_=rw[D: 2 * D, :])
            rwb_t = wp.tile([P, 2, D], bf16)
            nc.scalar.activation(out=rwb_t[:], in_=rwf[:], func=Act.Copy)

            # ---- P1 batches with stage-2 windows interleaved at cuts ----
            cuts = dims["cuts"]
            from contextlib import ExitStack
            with ExitStack() as stack:
                pool = lambda *a, **k: stack.enter_context(
                    tc.tile_pool(*a, **k))
                ip1 = pool(name="idx1", bufs=3)
                gp = pool(name="gat", bufs=2)
                tp = pool(name="xsT1", bufs=4)
                xp = pool(name="ex1", bufs=4)
                sm = pool(name="sm1", bufs=8)
                fsp = pool(name="fts", bufs=2)
                ip2 = pool(name="idx2", bufs=1)
                bg = pool(name="big2", bufs=1)
                wk2 = pool(name="wk2", bufs=10)
                xp2 = pool(name="ex2", bufs=10)
                sm2 = pool(name="sm2", bufs=8)
                psT = pool(name="psT", bufs=2, space="PSUM")
                psS = pool(name="psS", bufs=2, space="PSUM")
                psF = pool(name="psF", bufs=2, space="PSUM")
                ppA = pool(name="psA", bufs=1, space="PSUM")
                ppB = pool(name="psB", bufs=1, space="PSUM")
                ftgt = ip2.tile([P, NI // 16], i16, tag="ftg")
                nc.sync.dma_start(out=ftgt[:], in_=ftgd[:])

                def s1_batch(b):
                    xs = gp.tile([P, TPB, D + 1], bf16, tag="xs")
                    nc.sync.dma_start(out=xs[:], in_=xsd[b])
                    xdw0 = ip1.tile([P, UPB * SU], bf16, tag="xdw0")
                    nc.sync.dma_start(out=xdw0[:], in_=xdw0d[b])
                    xdw = ip1.tile([P, UPB * SU], bf16, tag="xdw")
                    nc.vector.tensor_scalar_mul(xdw[:], xdw0[:], pi_t[:])
                    b0t = ip1.tile([P, TPB * SU], bf16, tag="b0")
                    nc.sync.dma_start(out=b0t[:], in_=b0d[b])
                    ftstage = fsp.tile([P, BPB, D], bf16, tag="fts")
                    for blk in range(BPB):
                        # transpose this block's 8 edge tiles: [e,d] -> [d,e]
                        xsT = tp.tile([P, 8 * P], bf16, tag="xsT")
                        for g in range(2):
                            trp = psT.tile([P, 4 * P], bf16, space="PSUM",
                                           tag="tr")
                            for j in range(4):
                                t = blk * 8 + g * 4 + j
                                nc.tensor.transpose(
                                    out=trp[:, j * P: (j + 1) * P],
                                    in_=xs[:, t, 0:D], identity=ident_b[:])
                            dst = xsT[:, g * 4 * P: (g + 1) * 4 * P]
                            if (blk * 2 + g) % 2 == 0:
                                nc.scalar.activation(out=dst, in_=trp[:],
                                                     func=Act.Copy)
                            else:
                                nc.vector.tensor_copy(out=dst, in_=trp[:])
                        # narrow scores: S[e, 0:32] vs unit dst cols + bias
                        sp = psS.tile([P, 8 * SU], f32, space="PSUM",
                                      tag="sp")
                        for j in range(8):
                            t = blk * 8 + j
                            u = blk * 4 + j // 2       # unit within batch
                            nc.tensor.matmul(
                                out=sp[:, j * SU: (j + 1) * SU],
                                lhsT=xsT[:, j * P: (j + 1) * P],
                                rhs=xdw[:, u * SU: (u + 1) * SU],
                                start=True, stop=False)
                            nc.tensor.matmul(
                                out=sp[:, j * SU: (j + 1) * SU],
                                lhsT=ident_b[:],
                                rhs=b0t[:, t * SU: (t + 1) * SU],
                                start=False, stop=True)
                        exm = xp.tile([P, 8 * SU], bf16, tag="ex")
                        nc.scalar.activation(out=exm[:], in_=sp[:],
                                             func=Act.Exp)
                        # ft + den in one accumulation group per unit
                        fdp = psF.tile([P, D + 1], f32, space="PSUM",
                                       tag="ftden")
                        for j in range(8):
                            t = blk * 8 + j
                            off = SU * (j // 2)
                            nc.tensor.matmul(
                                out=fdp[off: off + SU, :],
                                lhsT=exm[:, j * SU: (j + 1) * SU],
                                rhs=xs[:, t, :],
                                start=(j % 2 == 0), stop=(j % 2 == 1),
                                tile_position=(0, off))
                        denc = sm.tile([P, 1], f32, tag="denc")
                        nc.vector.tensor_scalar_max(denc[:], fdp[:, D: D + 1],
                                                    1e-30)
                        rec = sm.tile([P, 1], f32, tag="rec")
                        nc.vector.reciprocal(rec[:], denc[:])
                        nc.vector.tensor_scalar_mul(
                            ftstage[:, blk, :], fdp[:, 0:D], rec[:])
                    nc.scalar.dma_start(
                        out=ftd[b * BPB * P: (b + 1) * BPB * P, :].rearrange(
                            "(g p) d -> p g d", p=P),
                        in_=ftstage[:])

                def s2_c0(w2, st2):
                    hpt = bg.tile([P, NW], bf16, tag="hpt")
                    nc.sync.dma_start(out=hpt[:],
                                      in_=hpT[:, w2 * NW: (w2 + 1) * NW])
                    tlt = sm2.tile([P, TI2], f32, tag="tlt")
                    nc.sync.dma_start(out=tlt[:], in_=tgtlocd[w2])
                    ftg = bg.tile([P, TI2, D], bf16, tag="ftgw")
                    ftgT = bg.tile([P, 1, NW], bf16, tag="ftgTw")
                    for o0, n in ((0, GH), (GH, NW - GH)):
                        o = w2 * NW + o0
                        nc.gpsimd.dma_gather(
                            out_ap=ftg[:, o0 // P: (o0 + n) // P, :],
                            in_ap=ftd[:],
                            idxs_ap=ftgt[:, o // 16: (o + n) // 16],
                            num_idxs=n, num_idxs_reg=n, elem_size=D,
                            single_packet=False)
                        nc.gpsimd.dma_gather(
                            out_ap=ftgT[:, :, o0: o0 + n],
                            in_ap=ftd[:],
                            idxs_ap=ftgt[:, o // 16: (o + n) // 16],
                            num_idxs=n, num_idxs_reg=n, elem_size=D,
                            single_packet=False, transpose=True)
                    st2.update(hpt=hpt, tlt=tlt, ftg=ftg, ftgT=ftgT)

                def s2_c1(w2, st2):
                    hpt, tlt, ftg, ftgT = (st2[k] for k in
                                           ("hpt", "tlt", "ftg", "ftgT"))
                    # one PSUM bank per window: mean | f | out, with
                    # strictly sequential accumulation groups (groups must
                    # not interleave within a PSUM bank)
                    apo = ppA.tile([P, 3 * D], f32, space="PSUM", tag="apo")
                    meanp = apo[:, 0:D]
                    fp = apo[:, D: 2 * D]
                    outp = apo[:, 2 * D: 3 * D]
                    rec2 = sm2.tile([P, 1], f32, tag="rec2")
                    nc.sync.dma_start(out=rec2[:], in_=rec2d[w2])
                    st2.update(apo=apo, meanp=meanp, fp=fp, outp=outp,
                               rec2=rec2)
                    # sweep A: mean (first half); deg comes from the host
                    for i in range(TI2 // 2):
                        mask = wk2.tile([P, P], bf16, tag="maskA")
                        nc.vector.tensor_scalar(
                            out=mask[:], in0=iota_t[:],
                            scalar1=tlt[:, i: i + 1], scalar2=None,
                            op0=Alu.is_equal)
                        nc.tensor.matmul(out=meanp, lhsT=mask[:],
                                         rhs=ftg[:, i, :],
                                         start=(i == 0), stop=(i == TI2 - 1))
                def s2_c2(w2, st2):
                    hpt, tlt, ftg, ftgT = (st2[k] for k in
                                           ("hpt", "tlt", "ftg", "ftgT"))
                    meanp, fp, rec2 = (st2[k] for k in
                                       ("meanp", "fp", "rec2"))
                    for i in range(TI2 // 2, TI2):
                        mask = wk2.tile([P, P], bf16, tag="maskA")
                        nc.vector.tensor_scalar(
                            out=mask[:], in0=iota_t[:],
                            scalar1=tlt[:, i: i + 1], scalar2=None,
                            op0=Alu.is_equal)
                        nc.tensor.matmul(out=meanp, lhsT=mask[:],
                                         rhs=ftg[:, i, :],
                                         start=(i == 0), stop=(i == TI2 - 1))
                    mean_sb = wk2.tile([P, D], f32, tag="mean_sb")
                    nc.vector.tensor_scalar_mul(mean_sb[:], meanp, rec2[:])
                    trx = ppB.tile([P, 4 * P], f32, space="PSUM", tag="big")
                    nc.tensor.transpose(out=trx[:, 0:P], in_=mean_sb[:],
                                        identity=ident_t[:])
                    meanT = wk2.tile([P, P], bf16, tag="meanT")
                    nc.scalar.activation(out=meanT[:], in_=trx[:, 0:P],
                                         func=Act.Copy)
                    htt = wk2.tile([P, P], bf16, tag="htt")
                    nc.sync.dma_start(out=htt[:],
                                      in_=htT[:, w2 * WS2: (w2 + 1) * WS2])
                    nc.tensor.matmul(out=fp, lhsT=htt[:], rhs=rwb_t[:, 0, :],
                                     start=True, stop=False)
                    nc.tensor.matmul(out=fp, lhsT=meanT[:],
                                     rhs=rwb_t[:, 1, :],
                                     start=False, stop=True)
                    # fT = f transposed [d, tgt] (for W = e2T^T @ fT)
                    f_sb = wk2.tile([P, D], f32, tag="f_sb")
                    nc.vector.tensor_copy(out=f_sb[:], in_=fp)
                    trf = ppB.tile([P, 4 * P], f32, space="PSUM", tag="big")
                    nc.tensor.transpose(out=trf[:, 0:P], in_=f_sb[:],
                                        identity=ident_t[:])
                    fTb = wk2.tile([P, P], bf16, tag="fTb")
                    nc.scalar.activation(out=fTb[:], in_=trf[:, 0:P],
                                         func=Act.Copy)
                    st2.update(fTb=fTb)

                def s2_swb(w2, st2, g0s):
                    hpt, tlt, ftg, ftgT, fTb, outp = (st2[k] for k in
                        ("hpt", "tlt", "ftg", "ftgT", "fTb", "outp"))
                    # sweep B: e2T = tanh(qw^T [ft, hp]^T); W = e2T^T fT;
                    # wc[islot] = W[islot, tl(islot)] via fused mask+reduce
                    for g0 in g0s:
                        gn = min(4, TI2 - g0)
                        e2p = ppB.tile([P, 4 * P], f32, space="PSUM",
                                       tag="big")
                        for j in range(gn):
                            i = g0 + j
                            nc.tensor.matmul(
                                out=e2p[:, j * P: (j + 1) * P],
                                lhsT=qwb_t[:, 0, :],
                                rhs=ftgT[:, 0, i * P: (i + 1) * P],
                                start=True, stop=False)
                            nc.tensor.matmul(
                                out=e2p[:, j * P: (j + 1) * P],
                                lhsT=qwb_t[:, 1, :],
                                rhs=hpt[:, i * P: (i + 1) * P],
                                start=False, stop=True)
                        e2T = xp2.tile([P, 4 * P], bf16, tag="e2sb")
                        nc.scalar.activation(out=e2T[:, : gn * P],
                                             in_=e2p[:, : gn * P],
                                             func=Act.Tanh)
                        wp_ = ppB.tile([P, 4 * P], f32, space="PSUM",
                                       tag="big")
                        for j in range(gn):
                            nc.tensor.matmul(
                                out=wp_[:, j * P: (j + 1) * P],
                                lhsT=e2T[:, j * P: (j + 1) * P],
                                rhs=fTb[:], start=True, stop=True)
                        for j in range(gn):
                            i = g0 + j
                            wsel = xp2.tile([P, P], bf16, tag="wsel")
                            wc = sm2.tile([P, 1], f32, tag="wc")
                            nc.vector.scalar_tensor_tensor(
                                out=wsel[:], in0=iota_t[:],
                                scalar=tlt[:, i: i + 1],
                                in1=wp_[:, j * P: (j + 1) * P],
                                op0=Alu.is_equal, op1=Alu.mult,
                                accum_out=wc[:])
                            maskw = wk2.tile([P, P], bf16, tag="maskw")
                            nc.vector.tensor_scalar(
                                out=maskw[:], in0=iota_t[:],
                                scalar1=tlt[:, i: i + 1],
                                scalar2=wc[:],
                                op0=Alu.is_equal, op1=Alu.mult)
                            nc.tensor.matmul(out=outp, lhsT=maskw[:],
                                             rhs=ftg[:, i, :],
                                             start=(i == 0),
                                             stop=(i == TI2 - 1))
                def s2_c3(w2, st2):
                    s2_swb(w2, st2, range(0, 24, 4))

                def s2_c4(w2, st2):
                    s2_swb(w2, st2, range(24, TI2, 4))
                    outp = st2["outp"]
                    out_sb = wk2.tile([P, D], f32, tag="out_sb")
                    nc.vector.tensor_copy(out=out_sb[:], in_=outp)
                    nc.scalar.dma_start(out=outd[w2 * WS2: (w2 + 1) * WS2, :],
                                        in_=out_sb[:])

                chunks = [s2_c0, s2_c1, s2_c2, s2_c3, s2_c4]
                sched = {}          # batch -> list of (chunk_fn, w2)
                states = [dict() for _ in range(W2)]
                for w2 in range(W2):
                    for ci, fn in enumerate(chunks):
                        sched.setdefault(cuts[w2] + ci, []).append((fn, w2))
                for b in range(B1):
                    s1_batch(b)
                    for fn, w2 in sched.get(b, []):
                        fn(w2, states[w2])
                for b in range(B1, max(sched) + 1):
                    for fn, w2 in sched.get(b, []):
                        fn(w2, states[w2])
    nc.compile()
    return nc


def make_in_maps(dims, cores, pi_w, q_w, r_w):
    iota_bf = np.tile(np.arange(P, dtype=np.float32), (P, 1)).astype(BF16)
    ident = np.eye(P, dtype=np.float32)
    in_maps = []
    for c in range(NCORES):
        st = cores[c]
        in_maps.append({
            "xsd": st["xs"],
            "xdw0d": st["xdw0"],
            "b0d": st["b0"],
            "hpT": st["hpT"],
            "htT": st["htT"],
            "qw": np.ascontiguousarray(q_w, np.float32),
            "rw": np.ascontiguousarray(r_w, np.float32),
            "pic": np.ascontiguousarray(pi_w.reshape(D, 1), np.float32),
            "iotab": iota_bf, "ident": ident,
            "ftgd": st["ftg"],
            "tgtlocd": st["tgtloc"], "rec2d": st["rec2"],
        })
    return in_maps


def unshard(dims, cores, results):
    NTGT = dims["NTGT"]
    out = np.zeros((NTGT, D), np.float32)
    for c in range(NCORES):
        st = cores[c]
        o = results[c]["out"]
        tw = st["twin"]
        for w2 in range(dims["W2"]):
            sel = tw[w2] >= 0
            out[tw[w2][sel]] = o[w2 * WS2: w2 * WS2 + WS2][sel]
    return out


def kernel(**inputs):
    from concourse.bass_utils import run_bass_kernel_spmd

    h_v = np.asarray(inputs["h_v"], np.float32)
    h_p = np.asarray(inputs["h_p"], np.float32)
    h_t = np.asarray(inputs["h_t"], np.float32)
    pi_w = np.asarray(inputs["pi_w"], np.float32)
    q_w = np.asarray(inputs["q_w"], np.float32)
    r_w = np.asarray(inputs["r_w"], np.float32)
    int_src = np.asarray(inputs["int_src"]).astype(np.int64)
    int_dst = np.asarray(inputs["int_dst"]).astype(np.int64)
    agg_src = np.asarray(inputs["agg_src"]).astype(np.int64)
    agg_dst = np.asarray(inputs["agg_dst"]).astype(np.int64)
    assert np.array_equal(agg_src, np.arange(agg_src.shape[0])), \
        "kernel assumes agg_src == arange (per problem spec fill)"

    dims, cores = preprocess(h_v, h_p, h_t, int_src, int_dst, agg_dst)
    nc = build_program(dims)
    global _LAST_NC
    _LAST_NC = nc
    in_maps = make_in_maps(dims, cores, pi_w, q_w, r_w)
    res = run_bass_kernel_spmd(nc, in_maps, core_ids=list(range(NCORES)))
    return unshard(dims, cores, res.results)


# revision 7
# speedup vs baseline: 1.4631x; 1.1512x over previous
"""Trainium2 Bass kernel for nn_DglAggregator (GNN message passing).

Strategy (8 NeuronCores, SPMD, one uniform program, per-core data):
- Targets are partitioned across cores balanced by stage-1 edge count; each
  core owns its targets' items and ALL stage-1 edges pointing at those items,
  so no cross-core communication is needed.
- One unified item-slot space shared by both stages: items (sorted by
  target) are packed into "units" of <=32 whole slots and <=256 edges
  (2 edge tiles of 128); 4 units form a "block" (128 slots); blocks are
  grouped into stage-2 windows (whole targets, <=128 targets) at block
  bases that are uniform across cores, and batches of 8 blocks (8192 edge
  ranks) for stage 1.
- Stage 1 (item->item segment softmax + weighted sum): the per-edge source
  rows arrive as ONE linear 2MB DMA per batch from a host-relaid
  edge-rank-ordered table (with a 129th all-ones column); no device
  gathers.  Narrow score tiles: S[e, 0:32] is computed against the unit's
  32 dst columns only (h_v[dst]*pi, streamed compactly); the segment mask
  arrives as a host-streamed bias table b0 (0 / -30000) added via a second
  accumulating matmul, so one batched exp per block yields masked softmax
  weights directly.  The ft matmul contracts edges with rhs [e,129] =
  [src_feat | 1], producing ft AND the softmax denominator in one
  accumulation group per unit at its partition offset; normalization
  happens in the PSUM->SBUF copy (per-partition reciprocal scale), which
  lands directly in a persistent per-window SBUF "ft image" - the ft table
  never touches DRAM.  Max-subtraction is skipped (|score| small).
- Stage 2 (item->target): reads ft directly from the SBUF image; the
  transposed orientation needed by the e2 matmuls is produced on-chip
  (PE transposes through PSUM, 4 blocks at a time).  e2 is computed in
  transposed orientation (lhsT = qw, rhs = ftT/hpT), the per-edge weight
  w = <e2, f[dst]> is a row-select of W = e2T^T fT via one fused
  scalar_tensor_tensor with accum_out.  Degree normalization is host graph
  metadata.  Stage-2 windows are emitted in chunks interleaved between
  stage-1 batches as soon as their image blocks exist (cuts); the final
  window is deliberately packed small to shorten the serial tail.
- Numeric tables (h_v/h_p/h_t) are staged in bfloat16; all arithmetic
  (pi scaling, matmuls, softmax, tanh, means) runs on the NeuronCores with
  f32 PSUM accumulation. Host work is index math, row permutation/layout
  of input tables, and dtype staging.

kernel(**inputs) accepts the FULL unsharded inputs and returns the FULL
[N_TGT, 128] float32 output.
"""
import numpy as np
import ml_dtypes

BF16 = np.dtype(ml_dtypes.bfloat16)

P = 128          # partitions / tile edge
D = 128          # feature dim
NCORES = 8
SU = 32          # slots per unit
EU = 256         # edge capacity per unit (2 tiles)
UPB = 32         # units per batch
TPB = 2 * UPB    # edge tiles per batch (64)
BPB = UPB // 4   # blocks per batch (8); block = 4 units = 128 slots
RB = TPB * P     # edge ranks per batch (8192)
W2CAP = 6400     # stage-2 window islot capacity
W2TAIL = 1280    # preferred size of the final (tail) window
WS2 = 128        # stage-2 window target capacity
_LAST_NC = None


def _interleave_f32(vals: np.ndarray, cap: int, fill: float) -> np.ndarray:
    """[n] -> [128, cap/128] f32 with value of rank r at [r%128, r//128]."""
    a = np.full(cap, fill, np.float32)
    a[: vals.shape[0]] = vals
    return a.reshape(cap // P, P).T.copy()


def _pack_runs(run_sizes, max_runs, max_total):
    """Greedy pack consecutive runs into groups of whole runs, <=max_runs
    runs and <=max_total total size. Returns list of (start_run, n_runs)."""
    groups = []
    i, n = 0, len(run_sizes)
    while i < n:
        tot, j = 0, i
        while j < n and j - i < max_runs and tot + run_sizes[j] <= max_total:
            tot += run_sizes[j]
            j += 1
        assert j > i, f"run {i} of size {run_sizes[i]} exceeds {max_total}"
        groups.append((i, j - i))
        i = j
    return groups


def _pack_units(degs, max_slots, max_edges):
    """Greedy pack consecutive items into units (<=max_slots whole items,
    <=max_edges edges). Returns (unit_of_item, slot_of_item, n_units)."""
    n = len(degs)
    unit = np.zeros(n, np.int64)
    slot = np.zeros(n, np.int64)
    u, ecnt, scnt = 0, 0, 0
    for q in range(n):
        d = int(degs[q])
        if ecnt + d > max_edges or scnt >= max_slots:
            u += 1
            ecnt, scnt = 0, 0
        unit[q] = u
        slot[q] = scnt
        ecnt += d
        scnt += 1
    return unit, slot, (u + 1) if n else 0


def preprocess(h_v, h_p, h_t, int_src, int_dst, agg_dst):
    """All graph restructuring. Returns shared dims + per-core arrays."""
    NITEM = h_v.shape[0]
    NTGT = h_t.shape[0]
    int_src = int_src.astype(np.int64)
    int_dst = int_dst.astype(np.int64)
    item_tgt = agg_dst.astype(np.int64)       # item i -> target (agg_src=arange)
    h_v_bf = h_v.astype(BF16)
    h_p_bf = h_p.astype(BF16)
    h_t_bf = h_t.astype(BF16)

    # ---- target -> core, balanced by stage-1 edge load ----
    deg_int = np.bincount(int_dst, minlength=NITEM)
    t_edges = np.bincount(item_tgt, weights=deg_int.astype(np.float64),
                          minlength=NTGT)
    t_items = np.bincount(item_tgt, minlength=NTGT)
    tgt_core = np.zeros(NTGT, np.int64)
    load = np.zeros(NCORES)
    for t in np.argsort(-t_edges, kind="stable"):
        c = int(np.argmin(load))
        tgt_core[t] = c
        load[c] += t_edges[t] + 0.5 * t_items[t]
    item_core = tgt_core[item_tgt]

    cores = []
    for c in range(NCORES):
        tlist = np.where(tgt_core == c)[0]
        items = np.where(item_core == c)[0]
        items = items[np.lexsort((items, item_tgt[items]))]
        cores.append({"targets": tlist, "items": items})

    # ---- stage-2 windows (whole targets); keep the final window small ----
    for c in range(NCORES):
        st = cores[c]
        sizes = t_items[st["targets"]]
        groups = _pack_runs(sizes, WS2, W2CAP)
        if len(groups) > 1 or (len(groups) == 1 and
                               sizes.sum() > W2TAIL + P):
            i0, n = groups[-1]
            tot = int(sizes[i0: i0 + n].sum())
            if tot > W2TAIL + P:
                # split the last group so the tail window is small
                acc, k = 0, n
                for j in range(n - 1, 0, -1):
                    acc += int(sizes[i0 + j])
                    if acc >= W2TAIL:
                        k = j
                        break
                if 0 < k < n:
                    groups[-1] = (i0, k)
                    groups.append((i0 + k, n - k))
        st["w2groups"] = groups
    W2 = max(len(st["w2groups"]) for st in cores)

    # ---- per-window unit packing; uniform block bases across cores ----
    for c in range(NCORES):
        st = cores[c]
        items = st["items"]
        degs = deg_int[items]
        tl = st["targets"]
        st["wininfo"] = []
        ip0 = 0
        for (t0, ntgt) in st["w2groups"]:
            nit = int(t_items[tl[t0: t0 + ntgt]].sum())
            unit, slot, nu = _pack_units(degs[ip0: ip0 + nit], SU, EU)
            st["wininfo"].append((ip0, nit, unit, slot, nu))
            ip0 += nit
        assert ip0 == len(items)
    NB2 = []
    for w2 in range(W2):
        nb = 1
        for c in range(NCORES):
            wi = cores[c]["wininfo"]
            if w2 < len(wi):
                nb = max(nb, (wi[w2][4] + 3) // 4)
        NB2.append(nb)
    WB2 = np.concatenate([[0], np.cumsum(NB2)]).astype(np.int64)
    NBLK = int(WB2[-1])
    B1 = (NBLK + BPB - 1) // BPB
    NBLK_PAD = B1 * BPB
    NISL = NBLK_PAD * P

    # ---- per-core unified slot assignment + edge-stream tables ----
    for c in range(NCORES):
        st = cores[c]
        items = st["items"]
        npos = len(items)
        unit_id = np.zeros(npos, np.int64)
        local_slot = np.zeros(npos, np.int64)
        tgtloc = np.full(npos, -1.0, np.float32)
        tslot = np.zeros(npos, np.int64)
        tl = st["targets"]
        twin = np.full((W2, WS2), -1, np.int64)
        for w2, (ip0, nit, unit, slot, nu) in enumerate(st["wininfo"]):
            unit_id[ip0: ip0 + nit] = WB2[w2] * 4 + unit
            local_slot[ip0: ip0 + nit] = slot
            t0, ntgt = st["w2groups"][w2]
            k = np.repeat(np.arange(ntgt), t_items[tl[t0: t0 + ntgt]])
            tgtloc[ip0: ip0 + nit] = k
            tslot[ip0: ip0 + nit] = w2 * WS2 + k
            twin[w2, :ntgt] = tl[t0: t0 + ntgt]
        st["twin"] = twin
        uslot = unit_id * SU + local_slot          # unified islot per item
        pos_of = np.full(NITEM, -1, np.int64)
        pos_of[items] = np.arange(npos)

        emask = item_core[int_dst] == c
        es = int_src[emask]
        epos = pos_of[int_dst[emask]]
        o = np.argsort(epos, kind="stable")
        es, epos = es[o], epos[o]
        ne = es.shape[0]

        eu = unit_id[epos]                       # unit of each edge
        ustart = np.zeros(NBLK_PAD * 4 + 1, np.int64)
        np.add.at(ustart, eu + 1, 1)
        ustart = np.cumsum(ustart)
        rank = np.arange(ne) - ustart[eu]        # rank within unit (<EU)
        assert rank.max(initial=0) < EU
        eb = eu // UPB                           # batch
        etile = (eu % UPB) * 2 + rank // P       # tile within batch
        ep = rank % P                            # partition row
        eseg = local_slot[epos]                  # unit-local slot

        xs = np.zeros((B1, P, TPB, D + 1), BF16)
        xs[eb, ep, etile, :D] = h_v_bf[es]
        xs[:, :, :, D] = 1.0
        b0 = np.full((B1, P, TPB * SU), -30000.0, BF16)
        b0[eb, ep, etile * SU + eseg] = 0.0
        st["xs"] = xs.reshape(B1, P, TPB * (D + 1))
        st["b0"] = b0

        # compact dst columns: unit u slot s -> h_v[item]
        xdw0 = np.zeros((B1, D, UPB * SU), BF16)
        qb = unit_id // UPB
        qcol = (unit_id % UPB) * SU + local_slot
        xdw0[qb, :, qcol] = h_v_bf[items]
        st["xdw0"] = xdw0

        # unified-layout stage-2 metadata
        tlf = np.full(NISL, -1.0, np.float32)
        tlf[uslot] = tgtloc
        st["tgtloc"] = tlf.reshape(NBLK_PAD, P).T.copy()   # [P, NBLK_PAD]
        hpTu = np.zeros((D, NISL), BF16)
        hpTu[:, uslot] = h_p_bf[items].T
        st["hpT"] = hpTu
        htT = np.zeros((D, W2 * WS2), BF16)
        tw = twin.reshape(-1)
        htT[:, tw >= 0] = h_t_bf[tw[tw >= 0]].T
        st["htT"] = htT
        r2 = np.ones((W2, P, 1), np.float32)
        for w2 in range(W2):
            sel = twin[w2] >= 0
            r2[w2, sel, 0] = 1.0 / np.maximum(t_items[twin[w2][sel]], 1)
        st["rec2"] = r2

    # earliest stage-1 batch after which each stage-2 window's blocks exist
    cuts = []
    for w2 in range(W2):
        lastu = (int(WB2[w2 + 1]) * 4) - 1
        cuts.append(min(lastu // UPB, B1 - 1))
    cuts = [max(cuts[: i + 1]) for i in range(W2)]
    cuts[W2 - 1] = B1 - 1

    dims = {"B1": B1, "W2": W2, "NB2": NB2, "WB2": WB2.tolist(),
            "NBLK": NBLK, "NBLK_PAD": NBLK_PAD, "cuts": cuts,
            "NITEM": NITEM, "NTGT": NTGT}
    return dims, cores


# ======================= device program =======================

def build_program(dims):
    import concourse.bacc as bacc
    import concourse.mybir as mybir
    import concourse.tile as tile

    f32 = mybir.dt.float32
    bf16 = mybir.dt.bfloat16
    Alu = mybir.AluOpType
    Act = mybir.ActivationFunctionType

    B1, W2, NBLK, NBLK_PAD = (dims[k] for k in
                              ("B1", "W2", "NBLK", "NBLK_PAD"))
    NB2, WB2 = dims["NB2"], dims["WB2"]
    NB2MAX = max(NB2)
    NISL = NBLK_PAD * P

    nc = bacc.Bacc("TRN2", target_bir_lowering=False, debug=False,
                   num_devices=NCORES)
    # inputs
    xsd = nc.dram_tensor("xsd", [B1, P, TPB * (D + 1)], bf16,
                         kind="ExternalInput")
    xdw0d = nc.dram_tensor("xdw0d", [B1, D, UPB * SU], bf16,
                           kind="ExternalInput")
    b0d = nc.dram_tensor("b0d", [B1, P, TPB * SU], bf16,
                         kind="ExternalInput")
    hpTd = nc.dram_tensor("hpTd", [D, NISL], bf16, kind="ExternalInput")
    htTd = nc.dram_tensor("htTd", [D, W2 * WS2], bf16, kind="ExternalInput")
    qw = nc.dram_tensor("qw", [2 * D, D], f32, kind="ExternalInput")
    rw = nc.dram_tensor("rw", [2 * D, D], f32, kind="ExternalInput")
    pic = nc.dram_tensor("pic", [D, 1], f32, kind="ExternalInput")
    iotab = nc.dram_tensor("iotab", [P, P], bf16, kind="ExternalInput")
    ident = nc.dram_tensor("ident", [P, P], f32, kind="ExternalInput")
    tgtlocd = nc.dram_tensor("tgtlocd", [P, NBLK_PAD], f32,
                             kind="ExternalInput")
    rec2d = nc.dram_tensor("rec2d", [W2, P, 1], f32, kind="ExternalInput")
    # output
    outd = nc.dram_tensor("out", [W2 * WS2, D], f32, kind="ExternalOutput")

    with tile.TileContext(nc) as tc:
        with (
            tc.tile_pool(name="consts", bufs=1) as cp,
            tc.tile_pool(name="weights", bufs=1) as wp,
            tc.tile_pool(name="img", bufs=1) as imgp,
        ):
            iota_t = cp.tile([P, P], bf16)
            nc.sync.dma_start(out=iota_t[:], in_=iotab[:])
            ident_t = cp.tile([P, P], f32)
            nc.sync.dma_start(out=ident_t[:], in_=ident[:])
            ident_b = cp.tile([P, P], bf16)
            nc.scalar.activation(out=ident_b[:], in_=ident_t[:], func=Act.Copy)
            pi_t = cp.tile([D, 1], f32)
            nc.sync.dma_start(out=pi_t[:], in_=pic[:])
            tltR = cp.tile([P, NBLK_PAD], f32)
            nc.sync.dma_start(out=tltR[:], in_=tgtlocd[:])
            # weights: load f32, cast to bf16 on device
            qwf = wp.tile([P, 2, D], f32)
            nc.sync.dma_start(out=qwf[:, 0, :], in_=qw[0:D, :])
            nc.sync.dma_start(out=qwf[:, 1, :], in_=qw[D: 2 * D, :])
            qwb_t = wp.tile([P, 2, D], bf16)
            nc.scalar.activation(out=qwb_t[:], in_=qwf[:], func=Act.Copy)
            rwf = wp.tile([P, 2, D], f32)
            nc.sync.dma_start(out=rwf[:, 0, :], in_=rw[0:D, :])
            nc.sync.dma_start(out=rwf[:, 1, :], in_=rw[D: 2 * D, :])
            rwb_t = wp.tile([P, 2, D], bf16)
            nc.scalar.activation(out=rwb_t[:], in_=rwf[:], func=Act.Copy)
            # persistent per-window ft image (the stage-1 output table)
            imgs = [imgp.tile([P, NB2[w2] * D], bf16, tag=f"img{w2}",
                              name=f"img{w2}")
                    for w2 in range(W2)]

            def blk_home(gblk):
                """global block -> (w2, image col base); None for pad."""
                if gblk >= NBLK:
                    return None
                w2 = int(np.searchsorted(WB2, gblk, side="right")) - 1
                return w2, (gblk - WB2[w2]) * D

            # ---- P1 batches with stage-2 windows interleaved at cuts ----
            cuts = dims["cuts"]
            from contextlib import ExitStack
            with ExitStack() as stack:
                pool = lambda *a, **k: stack.enter_context(
                    tc.tile_pool(*a, **k))
                ip1 = pool(name="idx1", bufs=3)
                gp = pool(name="gat", bufs=3)
                tp = pool(name="xsT1", bufs=4)
                xp = pool(name="ex1", bufs=4)
                sm = pool(name="sm1", bufs=8)
                bg = pool(name="big2", bufs=1)
                wk2 = pool(name="wk2", bufs=10)
                xp2 = pool(name="ex2", bufs=10)
                sm2 = pool(name="sm2", bufs=8)
                psT = pool(name="psT", bufs=2, space="PSUM")
                psS = pool(name="psS", bufs=2, space="PSUM")
                psF = pool(name="psF", bufs=2, space="PSUM")
                ppA = pool(name="psA", bufs=1, space="PSUM")
                ppB = pool(name="psB", bufs=1, space="PSUM")

                def s1_batch(b):
                    nblk = min(BPB, NBLK - b * BPB)
                    xs = gp.tile([P, TPB, D + 1], bf16, tag="xs")
                    nc.sync.dma_start(out=xs[:], in_=xsd[b])
                    xdw0 = ip1.tile([P, UPB * SU], bf16, tag="xdw0")
                    nc.sync.dma_start(out=xdw0[:], in_=xdw0d[b])
                    xdw = ip1.tile([P, UPB * SU], bf16, tag="xdw")
                    nc.vector.tensor_scalar_mul(xdw[:], xdw0[:], pi_t[:])
                    b0t = ip1.tile([P, TPB * SU], bf16, tag="b0")
                    nc.sync.dma_start(out=b0t[:], in_=b0d[b])
                    for blk in range(nblk):
                        w2i, col = blk_home(b * BPB + blk)
                        # transpose this block's 8 edge tiles: [e,d] -> [d,e]
                        xsT = tp.tile([P, 8 * P], bf16, tag="xsT")
                        for g in range(2):
                            trp = psT.tile([P, 4 * P], bf16, space="PSUM",
                                           tag="tr")
                            for j in range(4):
                                t = blk * 8 + g * 4 + j
                                nc.tensor.transpose(
                                    out=trp[:, j * P: (j + 1) * P],
                                    in_=xs[:, t, 0:D], identity=ident_b[:])
                            dst = xsT[:, g * 4 * P: (g + 1) * 4 * P]
                            if (blk * 2 + g) % 2 == 0:
                                nc.scalar.activation(out=dst, in_=trp[:],
                                                     func=Act.Copy)
                            else:
                                nc.vector.tensor_copy(out=dst, in_=trp[:])
                        # narrow scores: S[e, 0:32] vs unit dst cols + bias
                        sp = psS.tile([P, 8 * SU], f32, space="PSUM",
                                      tag="sp")
                        for j in range(8):
                            t = blk * 8 + j
                            u = blk * 4 + j // 2       # unit within batch
                            nc.tensor.matmul(
                                out=sp[:, j * SU: (j + 1) * SU],
                                lhsT=xsT[:, j * P: (j + 1) * P],
                                rhs=xdw[:, u * SU: (u + 1) * SU],
                                start=True, stop=False)
                            nc.tensor.matmul(
                                out=sp[:, j * SU: (j + 1) * SU],
                                lhsT=ident_b[:],
                                rhs=b0t[:, t * SU: (t + 1) * SU],
                                start=False, stop=True)
                        exm = xp.tile([P, 8 * SU], bf16, tag="ex")
                        nc.scalar.activation(out=exm[:], in_=sp[:],
                                             func=Act.Exp)
                        # ft + den in one accumulation group per unit
                        fdp = psF.tile([P, D + 1], f32, space="PSUM",
                                       tag="ftden")
                        for j in range(8):
                            t = blk * 8 + j
                            off = SU * (j // 2)
                            nc.tensor.matmul(
                                out=fdp[off: off + SU, :],
                                lhsT=exm[:, j * SU: (j + 1) * SU],
                                rhs=xs[:, t, :],
                                start=(j % 2 == 0), stop=(j % 2 == 1),
                                tile_position=(0, off))
                        denc = sm.tile([P, 1], f32, tag="denc")
                        nc.vector.tensor_scalar_max(denc[:], fdp[:, D: D + 1],
                                                    1e-30)
                        rec = sm.tile([P, 1], f32, tag="rec")
                        nc.vector.reciprocal(rec[:], denc[:])
                        nc.vector.tensor_scalar_mul(
                            imgs[w2i][:, col: col + D], fdp[:, 0:D], rec[:])

                def s2_c0(w2, st2):
                    nb = NB2[w2]
                    hpt = bg.tile([P, NB2MAX * P], bf16, tag="hpt")
                    nc.sync.dma_start(
                        out=hpt[:, : nb * P],
                        in_=hpTd[:, WB2[w2] * P: (WB2[w2] + nb) * P])
                    rec2 = sm2.tile([P, 1], f32, tag="rec2")
                    nc.sync.dma_start(out=rec2[:], in_=rec2d[w2])
                    st2.update(hpt=hpt, rec2=rec2, img=imgs[w2],
                               tb=WB2[w2], nb=nb)

                def s2_c1(w2, st2):
                    img, tb, nb, rec2 = (st2[k] for k in
                                         ("img", "tb", "nb", "rec2"))
                    # one PSUM bank per window: mean | f | out, with
                    # strictly sequential accumulation groups (groups must
                    # not interleave within a PSUM bank)
                    apo = ppA.tile([P, 3 * D], f32, space="PSUM", tag="apo")
                    meanp = apo[:, 0:D]
                    fp = apo[:, D: 2 * D]
                    outp = apo[:, 2 * D: 3 * D]
                    st2.update(apo=apo, meanp=meanp, fp=fp, outp=outp)
                    # sweep A: mean (first half); deg comes from the host
                    for i in range(nb // 2):
                        mask = wk2.tile([P, P], bf16, tag="maskA")
                        nc.vector.tensor_scalar(
                            out=mask[:], in0=iota_t[:],
                            scalar1=tltR[:, tb + i: tb + i + 1], scalar2=None,
                            op0=Alu.is_equal)
                        nc.tensor.matmul(out=meanp, lhsT=mask[:],
                                         rhs=img[:, i * D: (i + 1) * D],
                                         start=(i == 0), stop=(i == nb - 1))

                def s2_c2(w2, st2):
                    img, tb, nb, rec2, meanp, fp = (st2[k] for k in
                        ("img", "tb", "nb", "rec2", "meanp", "fp"))
                    for i in range(nb // 2, nb):
                        mask = wk2.tile([P, P], bf16, tag="maskA")
                        nc.vector.tensor_scalar(
                            out=mask[:], in0=iota_t[:],
                            scalar1=tltR[:, tb + i: tb + i + 1], scalar2=None,
                            op0=Alu.is_equal)
                        nc.tensor.matmul(out=meanp, lhsT=mask[:],
                                         rhs=img[:, i * D: (i + 1) * D],
                                         start=(i == 0), stop=(i == nb - 1))
                    mean_sb = wk2.tile([P, D], f32, tag="mean_sb")
                    nc.vector.tensor_scalar_mul(mean_sb[:], meanp, rec2[:])
                    trx = ppB.tile([P, 4 * P], f32, space="PSUM", tag="big")
                    nc.tensor.transpose(out=trx[:, 0:P], in_=mean_sb[:],
                                        identity=ident_t[:])
                    meanT = wk2.tile([P, P], bf16, tag="meanT")
                    nc.scalar.activation(out=meanT[:], in_=trx[:, 0:P],
                                         func=Act.Copy)
                    htt = wk2.tile([P, P], bf16, tag="htt")
                    nc.sync.dma_start(out=htt[:],
                                      in_=htTd[:, w2 * WS2: (w2 + 1) * WS2])
                    nc.tensor.matmul(out=fp, lhsT=htt[:], rhs=rwb_t[:, 0, :],
                                     start=True, stop=False)
                    nc.tensor.matmul(out=fp, lhsT=meanT[:],
                                     rhs=rwb_t[:, 1, :],
                                     start=False, stop=True)
                    # fT = f transposed [d, tgt] (for W = e2T^T @ fT)
                    f_sb = wk2.tile([P, D], f32, tag="f_sb")
                    nc.vector.tensor_copy(out=f_sb[:], in_=fp)
                    trf = ppB.tile([P, 4 * P], f32, space="PSUM", tag="big")
                    nc.tensor.transpose(out=trf[:, 0:P], in_=f_sb[:],
                                        identity=ident_t[:])
                    fTb = wk2.tile([P, P], bf16, tag="fTb")
                    nc.scalar.activation(out=fTb[:], in_=trf[:, 0:P],
                                         func=Act.Copy)
                    st2.update(fTb=fTb)

                def s2_swb(w2, st2, g0s):
                    img, tb, nb, hpt, fTb, outp = (st2[k] for k in
                        ("img", "tb", "nb", "hpt", "fTb", "outp"))
                    # sweep B: ftT on-chip; e2T = tanh(qw^T [ft, hp]^T);
                    # W = e2T^T fT; wc = W[islot, tl(islot)] via fused
                    # scalar_tensor_tensor with accum_out
                    for g0 in g0s:
                        gn = min(4, nb - g0)
                        trp2 = psT.tile([P, 4 * P], bf16, space="PSUM",
                                        tag="tr")
                        for j in range(gn):
                            i = g0 + j
                            nc.tensor.transpose(
                                out=trp2[:, j * P: (j + 1) * P],
                                in_=img[:, i * D: (i + 1) * D],
                                identity=ident_b[:])
                        ftT = xp2.tile([P, 4 * P], bf16, tag="ftT")
                        if g0 % 8 == 0:
                            nc.scalar.activation(out=ftT[:, : gn * P],
                                                 in_=trp2[:, : gn * P],
                                                 func=Act.Copy)
                        else:
                            nc.vector.tensor_copy(out=ftT[:, : gn * P],
                                                  in_=trp2[:, : gn * P])
                        e2p = ppB.tile([P, 4 * P], f32, space="PSUM",
                                       tag="big")
                        for j in range(gn):
                            i = g0 + j
                            nc.tensor.matmul(
                                out=e2p[:, j * P: (j + 1) * P],
                                lhsT=qwb_t[:, 0, :],
                                rhs=ftT[:, j * P: (j + 1) * P],
                                start=True, stop=False)
                            nc.tensor.matmul(
                                out=e2p[:, j * P: (j + 1) * P],
                                lhsT=qwb_t[:, 1, :],
                                rhs=hpt[:, i * P: (i + 1) * P],
                                start=False, stop=True)
                        e2T = xp2.tile([P, 4 * P], bf16, tag="e2sb")
                        nc.scalar.activation(out=e2T[:, : gn * P],
                                             in_=e2p[:, : gn * P],
                                             func=Act.Tanh)
                        wp_ = ppB.tile([P, 4 * P], f32, space="PSUM",
                                       tag="big")
                        for j in range(gn):
                            nc.tensor.matmul(
                                out=wp_[:, j * P: (j + 1) * P],
                                lhsT=e2T[:, j * P: (j + 1) * P],
                                rhs=fTb[:], start=True, stop=True)
                        for j in range(gn):
                            i = g0 + j
                            wsel = xp2.tile([P, P], bf16, tag="wsel")
                            wc = sm2.tile([P, 1], f32, tag="wc")
                            nc.vector.scalar_tensor_tensor(
                                out=wsel[:], in0=iota_t[:],
                                scalar=tltR[:, tb + i: tb + i + 1],
                                in1=wp_[:, j * P: (j + 1) * P],
                                op0=Alu.is_equal, op1=Alu.mult,
                                accum_out=wc[:])
                            maskw = wk2.tile([P, P], bf16, tag="maskw")
                            nc.vector.tensor_scalar(
                                out=maskw[:], in0=iota_t[:],
                                scalar1=tltR[:, tb + i: tb + i + 1],
                                scalar2=wc[:],
                                op0=Alu.is_equal, op1=Alu.mult)
                            nc.tensor.matmul(out=outp, lhsT=maskw[:],
                                             rhs=img[:, i * D: (i + 1) * D],
                                             start=(i == 0),
                                             stop=(i == nb - 1))

                def s2_c3(w2, st2):
                    g0s = list(range(0, st2["nb"], 4))
                    s2_swb(w2, st2, g0s[: len(g0s) // 2])

                def s2_c4(w2, st2):
                    g0s = list(range(0, st2["nb"], 4))
                    s2_swb(w2, st2, g0s[len(g0s) // 2:])
                    outp = st2["outp"]
                    out_sb = wk2.tile([P, D], f32, tag="out_sb")
                    nc.vector.tensor_copy(out=out_sb[:], in_=outp)
                    nc.scalar.dma_start(out=outd[w2 * WS2: (w2 + 1) * WS2, :],
                                        in_=out_sb[:])

                chunks = [s2_c0, s2_c1, s2_c2, s2_c3, s2_c4]
                # Global monotone chunk order: windows in order, chunks in
                # order, batch positions non-decreasing.  This guarantees a
                # window's first use of the bufs=1 resources (hpt, apo) sits
                # AFTER the previous window's last use in program order, so
                # pool rotation can never create an engine-order cycle
                # (deadlock) when late windows' cut batches are close.
                sched = {}          # batch -> list of (chunk_fn, w2)
                states = [dict() for _ in range(W2)]
                pos = 0
                for w2 in range(W2):
                    bhalf = (WB2[w2] + NB2[w2] // 2 - 1) // BPB
                    dcs = [cuts[w2] - 3, max(bhalf, cuts[w2] - 2),
                           cuts[w2], cuts[w2] + 1, cuts[w2] + 2]
                    for ci, fn in enumerate(chunks):
                        pos = max(pos, dcs[ci], 0)
                        sched.setdefault(pos, []).append((fn, w2))
                for b in range(B1):
                    s1_batch(b)
                    for fn, w2 in sched.get(b, []):
                        fn(w2, states[w2])
                for b in range(B1, max(sched) + 1):
                    for fn, w2 in sched.get(b, []):
                        fn(w2, states[w2])
    nc.compile()
    return nc


def make_in_maps(dims, cores, pi_w, q_w, r_w):
    iota_bf = np.tile(np.arange(P, dtype=np.float32), (P, 1)).astype(BF16)
    ident = np.eye(P, dtype=np.float32)
    in_maps = []
    for c in range(NCORES):
        st = cores[c]
        in_maps.append({
            "xsd": st["xs"],
            "xdw0d": st["xdw0"],
            "b0d": st["b0"],
            "hpTd": st["hpT"],
            "htTd": st["htT"],
            "qw": np.ascontiguousarray(q_w, np.float32),
            "rw": np.ascontiguousarray(r_w, np.float32),
            "pic": np.ascontiguousarray(pi_w.reshape(D, 1), np.float32),
            "iotab": iota_bf, "ident": ident,
            "tgtlocd": st["tgtloc"], "rec2d": st["rec2"],
        })
    return in_maps


def unshard(dims, cores, results):
    NTGT = dims["NTGT"]
    out = np.zeros((NTGT, D), np.float32)
    for c in range(NCORES):
        st = cores[c]
        o = results[c]["out"]
        tw = st["twin"]
        for w2 in range(dims["W2"]):
            sel = tw[w2] >= 0
            out[tw[w2][sel]] = o[w2 * WS2: w2 * WS2 + WS2][sel]
    return out


def kernel(**inputs):
    from concourse.bass_utils import run_bass_kernel_spmd

    h_v = np.asarray(inputs["h_v"], np.float32)
    h_p = np.asarray(inputs["h_p"], np.float32)
    h_t = np.asarray(inputs["h_t"], np.float32)
    pi_w = np.asarray(inputs["pi_w"], np.float32)
    q_w = np.asarray(inputs["q_w"], np.float32)
    r_w = np.asarray(inputs["r_w"], np.float32)
    int_src = np.asarray(inputs["int_src"]).astype(np.int64)
    int_dst = np.asarray(inputs["int_dst"]).astype(np.int64)
    agg_src = np.asarray(inputs["agg_src"]).astype(np.int64)
    agg_dst = np.asarray(inputs["agg_dst"]).astype(np.int64)
    assert np.array_equal(agg_src, np.arange(agg_src.shape[0])), \
        "kernel assumes agg_src == arange (per problem spec fill)"

    dims, cores = preprocess(h_v, h_p, h_t, int_src, int_dst, agg_dst)
    nc = build_program(dims)
    global _LAST_NC
    _LAST_NC = nc
    in_maps = make_in_maps(dims, cores, pi_w, q_w, r_w)
    res = run_bass_kernel_spmd(nc, in_maps, core_ids=list(range(NCORES)))
    return unshard(dims, cores, res.results)
